# revision 5
# baseline (speedup 1.0000x reference)
"""Trainium2 Bass kernel for nn_BondLenConstrain (peptide-bond gaussian NLL).

Contract: kernel(**inputs) takes the FULL unsharded inputs (as produced by
reference.setup_inputs()) and returns the FULL [B, CH, R, NALT] output.

Strategy (v5)
-------------
Structured input layout (verified on host): atom index = ((b*CH+ch)*R + r)*3
+ at, every bond valid; mean/std rows identical -> the gaussian NLL folds to
per-feature clamped parabolas  score_f = min((a_f*x_f + b_f)^2, C_f).

Data-parallel over batch: core i handles batches [2i, 2i+2).  Per batch the
coords are loaded as overlapped 585-float partition rows (residues
[64p, 64p+64] inclusive), pre-scaled by 1/8 on the host so all fp16
intermediates stay in range.  Per group (batch):
  D1  = S[j+3]-S[j]            fp32 sub -> fp16 diffs (all atom-diff comps)
  N   = sq+sq / sq+acc customs -> squared norms of every diff (no Act square)
  DT  = 3 strided products + 2 dense fp16 adds -> both dot planes
  PCP = N[3k+1+t]*N[3k+2+t]    norm-product planes
  QQ  = PCP - DT^2 (custom, fp32)  ->  RQ = abs_rsqrt (Act)
  TC  = DT*RQ  ->  AR = arctan (Act, single table switch for whole kernel)
  ACC = min((na2*rna*a0'+b0)^2,C0) then two chained WMIN adds  (fp16)
Output: compact fp16 per-bond scores DMA'd out per group as soon as ready;
the [B,CH,R,NALT] slab is assembled host-side during unshard.
"""

import numpy as np

B, CH, R, NALT = 16, 8, 8192, 10
EPS = 1e-10
NCORES = 8
BPC = B // NCORES            # batches per core = 2
K = 64                       # residues per partition (128*64 = 8192 = R)
CW = 9 * K + 9               # loaded floats per chain-row = 585 (overlap 9)
TW = 582                     # D1 width per chain
NW = 194                     # norms per chain-row (diff vectors)
CHAIN_F = R * 9              # floats per chain = 73728
GRP_F = CH * CHAIN_F         # floats per batch = 589824
CORE_F = BPC * GRP_F         # coords floats per core = 1179648
DEG = 180.0 / np.pi
PRE = 0.125                  # host-side coordinate pre-scale

_BUILT = {}


# ---------------------------------------------------------------- custom ops
def _register_dve_ops():
    import concourse.dve_ops as dvo
    from concourse.dve_spec import (
        Spec, Src0, Src1, C0, C1, C2, lower, maxx, minn, sq, _has_src1,
    )
    from concourse.dve_uop import DveOpSpec

    def mk(name, spec):
        for o in dvo.OPS:
            if o.name == name:
                return o
        row = dvo._CUSTOM_DVE_ROW_BASE + len(dvo.OPS)
        assert row < 0x20, "custom DVE op rows exhausted"
        shas = {}
        for ver in ("v3", "v4"):
            u = lower(spec, ver=ver)
            shas[ver] = DveOpSpec(
                name=name, opcode=row, uops=u, rd1_en=_has_src1(spec)
            ).sha(ver)
        op = dvo.DveOp(name, spec, subdim=False, uops_sha=shas)
        dvo.OPS.append(op)
        dvo.CUSTOM_DVE_SPECS[name] = spec
        dvo._SUB_OPCODE_FOR_NAME[name] = row
        return op

    ops = {}
    # n = x^2 + y^2
    ops["NRMA"] = mk("ANT_BL_NRMA", Spec(
        body=sq(Src0) + sq(Src1),
        reference=lambda in0, in1, s0, s1, imm2:
            (in0.astype(np.float32) ** 2 + in1.astype(np.float32) ** 2
             ).astype(np.float32),
    ))
    # n = x^2 + acc
    ops["NRMB"] = mk("ANT_BL_NRMB", Spec(
        body=sq(Src0) + Src1,
        reference=lambda in0, in1, s0, s1, imm2:
            (in0.astype(np.float32) ** 2 + in1.astype(np.float32)
             ).astype(np.float32),
    ))
    # q = max(pcat - dot^2, eps)
    ops["QSUB"] = mk("ANT_BL_QSUB", Spec(
        body=maxx(Src0 - sq(Src1 * C1), C0),
        reference=lambda in0, in1, s0, s1, imm2:
            np.maximum(in0 - (in1.astype(np.float32) * s1) ** 2, s0
                       ).astype(np.float32),
    ))
    # acc0 = min((na2*rna*a + b)^2, C)   (blen = na2*rna)
    ops["WMIN0B"] = mk("ANT_BL_WMIN0B", Spec(
        body=minn(sq(Src0 * Src1 * C0 + C1), C2),
        reference=lambda in0, in1, s0, s1, imm2:
            np.minimum((in0 * in1 * s0 + s1).astype(np.float32) ** 2, imm2
                       ).astype(np.float32),
    ))
    # acc = min((x*a + b)^2, C) + acc_prev
    ops["WMIN"] = mk("ANT_BL_WMIN", Spec(
        body=minn(sq(Src0 * C0 + C1), C2) + Src1,
        reference=lambda in0, in1, s0, s1, imm2:
            (np.minimum((in0 * s0 + s1).astype(np.float32) ** 2, imm2) + in1
             ).astype(np.float32),
    ))
    return ops


# ------------------------------------------------------------- host helpers
def _check_structured(atom_description, coords, mean, std, weight):
    if atom_description.shape != (B * CH * R * 3, 5):
        return False
    if coords.shape != (B * CH * R * 3, 3):
        return False
    if mean.shape != (20, 3) or std.shape != (20, 3) or weight.shape != (1,):
        return False
    if not ((mean == mean[0]).all() and (std == std[0]).all()):
        return False
    ad = atom_description
    n = B * CH * R
    at = np.tile(np.array([0, 1, 2], dtype=ad.dtype), n)
    if not np.array_equal(ad[:, 0], at):
        return False
    r = np.repeat(np.tile(np.arange(R, dtype=ad.dtype), B * CH), 3)
    if not np.array_equal(ad[:, 1], r):
        return False
    c = np.repeat(np.tile(np.arange(CH, dtype=ad.dtype), B), R * 3)
    if not np.array_equal(ad[:, 2], c):
        return False
    b = np.repeat(np.arange(B, dtype=ad.dtype), CH * R * 3)
    if not np.array_equal(ad[:, 3], b):
        return False
    return True


def _consts(mean, std, weight):
    """Fold mean/std/weight into per-feature device constants."""
    mu = mean[0].astype(np.float64)        # [3]
    var = std[0].astype(np.float64) ** 2   # [3]
    denom = np.sqrt(2.0 * np.pi * var)
    scale = float(1.0 - np.tanh(-np.float64(weight[0])))
    hiv = scale / (2.0 * var)
    Cs = (-np.log(EPS) - np.log(denom)) * scale
    a0 = np.sqrt(hiv[0])
    b0 = -mu[0] * a0
    a1 = -DEG * np.sqrt(hiv[1])
    b1 = (DEG * np.pi / 2.0 - mu[1]) * np.sqrt(hiv[1])
    a2 = -DEG * np.sqrt(hiv[2])
    b2 = (DEG * np.pi / 2.0 - mu[2]) * np.sqrt(hiv[2])
    # sanity: the clamp band must sit inside (0, 180) so that the arctan
    # formulation (t = cot(ang), saturating table) covers it
    d1 = np.sqrt(Cs[1] / hiv[1])
    d2 = np.sqrt(Cs[2] / hiv[2])
    ang_lo = max(min(mu[1] - d1, mu[2] - d2), 0.0)
    ang_hi = min(max(mu[1] + d1, mu[2] + d2), 180.0)
    if not (0.0 < ang_lo and ang_hi < 180.0):
        return None
    vals = [a0, b0, Cs[0], a1, b1, Cs[1], a2, b2, Cs[2]]
    return tuple(np.float32(v) for v in vals)


# ------------------------------------------------------------------- device
GP_DADD = False    # dot-plane adds on GpSimd instead of DVE
GP_PC = False      # norm-product planes on GpSimd instead of DVE


def _build(consts):
    import concourse.bacc as bacc
    import concourse.bass as bass
    import concourse.mybir as mybir
    from concourse.alu_op_type import AluOpType as alu
    from concourse.tile import TileContext

    OPS = _register_dve_ops()
    a0, b0, C0, a1, b1, C1, a2, b2, C2 = (float(v) for v in consts)
    a0s = a0 / PRE             # blen path sees blen*PRE
    f32 = mybir.dt.float32
    f16 = mybir.dt.float16
    AF = mybir.ActivationFunctionType

    nc = bacc.Bacc("TRN2", target_bir_lowering=False, debug=False)
    # sarr = coords repacked host-side (and pre-scaled by PRE) into the SBUF
    # slab layout [g][p][c][j(585)], so every load chunk is contiguous rows
    SPW = CH * CW  # per-partition floats per group = 4680
    sarr = nc.dram_tensor("sarr", [BPC * 128 * SPW], f32, kind="ExternalInput")
    out = nc.dram_tensor("out", [BPC * 128 * CH * K], f16,
                         kind="ExternalOutput")

    GNB = CH * K  # bonds per partition per group = 512
    with TileContext(nc) as tc:
        with (
            tc.tile_pool(name="io", bufs=1) as io,
            tc.tile_pool(name="wk", bufs=1) as wk,
        ):
            # preload the abs_rsqrt activation table during the DMA wait
            scr = wk.tile([128, 8], f32, tag="scr")
            nc.vector.memset(scr[:], 0.0)
            nc.scalar.activation(scr[:, 0:1], scr[:, 0:1], AF.Abs_reciprocal_sqrt)
            # single gate, chain-written after every abs_rsqrt op (below)
            gate = wk.tile([128, 1], f32, tag="gate")

            # units taper [12, 4]: the tail-critical last unit is small.
            # unit u covers chains [coff, coff+cn) of the flat 16-chain
            # per-core layout ([batch][chain] major); DMA per batch segment
            units = []
            for u, (coff, ucn, chunks) in enumerate(
                    ((0, 8, [1, 2, 2, 3]), (8, 8, [2, 3, 3]))):
                S = io.tile([128, ucn * CW], f32, tag=f"S{u}")
                c0 = 0
                for cn in chunks:
                    # source: chain c lives at batch c//CH, col c%CH
                    cg = coff + c0
                    g0_, ci = divmod(cg, CH)
                    assert ci + cn <= CH or g0_ * CH + ci + cn <= 16
                    # a chunk never crosses a batch boundary by construction
                    assert (ci + cn) <= CH
                    nc.sync.dma_start(
                        S[:, c0 * CW:(c0 + cn) * CW],
                        bass.AP(sarr, g0_ * 128 * SPW + ci * CW,
                                [[SPW, 128], [1, cn * CW]]),
                    )
                    c0 += cn
                units.append((u, coff, ucn, S, chunks))

            # D1 = S[j+3]-S[j] for all units first (DVE fp32 1x;
            # chunk-aligned so diffs start as soon as each chunk lands)
            d1s = {}
            for u, coff, ucn, S, chunks in units:
                D1 = wk.tile([128, ucn * TW], f32, tag=f"D1{u}")
                d1s[u] = D1

                def sv(off, c0, cn):
                    return bass.AP(S.tensor, S.offset + c0 * CW + off,
                                   [S.ap[0], [CW, cn], [195, 3], [1, NW]])

                def dv(c0, cn):
                    return bass.AP(D1.tensor, D1.offset + c0 * TW,
                                   [D1.ap[0], [TW, cn], [1, TW]])

                c0 = 0
                for cn in chunks:
                    nc.vector.tensor_tensor(
                        dv(c0, cn), sv(1, c0, cn), sv(0, c0, cn), alu.subtract)
                    c0 += cn

            tcs, accs = {}, {}
            for u, coff, ucn, S, chunks in units:
                D1 = d1s[u]
                UNB = ucn * K
                SQ = wk.tile([128, ucn * TW], f16, tag=f"SQ{u}")
                N = wk.tile([128, ucn * NW], f16, tag=f"N{u}")
                MTB = wk.tile([128, 2 * ucn * 3 * K], f16, tag=f"MTB{u}")
                TMD = wk.tile([128, 2 * UNB], f16, tag=f"TMD{u}")
                DT = wk.tile([128, 2 * UNB], f16, tag=f"DT{u}")
                PCP = wk.tile([128, 2 * UNB], f32, tag=f"PCP{u}")
                QQ = wk.tile([128, 2 * UNB], f32, tag=f"QQ{u}")
                RQ = wk.tile([128, 2 * UNB], f16, tag=f"RQ{u}")
                RN = wk.tile([128, UNB], f32, tag=f"RN{u}")
                TC = wk.tile([128, 2 * UNB], f16, tag=f"TC{u}")
                ACC = wk.tile([128, UNB], f16, tag=f"ACC{u}")
                tcs[u], accs[u] = TC, ACC

                def d1v(off, inner):
                    return bass.AP(D1.tensor, D1.offset + off,
                                   [D1.ap[0], [TW, ucn]] + inner)

                def nv(off, inner):
                    return bass.AP(N.tensor, N.offset + off,
                                   [N.ap[0], [NW, ucn]] + inner)

                # squares on Act (fp32 dense in, fp16 out; square is in
                # every table set so no extra table load), written comp-
                # planar [c][c3][a] so the sum3 adds run dense at 2x
                def sqp(c0, cn, c3):
                    return bass.AP(SQ.tensor, SQ.offset + c0 * TW + c3 * NW,
                                   [SQ.ap[0], [TW, cn], [1, NW]])

                def nh(c0, cn):
                    return bass.AP(N.tensor, N.offset + c0 * NW,
                                   [N.ap[0], [NW, cn], [1, NW]])

                # split on the 2nd DMA chunk boundary so the first half's
                # squares+norm sums pipeline with the last SUB chunk
                h0 = min(chunks[0] + chunks[1], ucn)
                for c0, cn in ((0, h0), (h0, ucn - h0)):
                    if cn <= 0:
                        continue
                    nc.scalar.activation(
                        SQ[:, c0 * TW:(c0 + cn) * TW],
                        D1[:, c0 * TW:(c0 + cn) * TW], AF.Square)
                    nc.vector.tensor_tensor(
                        nh(c0, cn), sqp(c0, cn, 0), sqp(c0, cn, 1), alu.add)
                    nc.vector.tensor_tensor(
                        nh(c0, cn), nh(c0, cn), sqp(c0, cn, 2), alu.add)

                # dot products, plane-major [t][c][k][c3], unit inner runs:
                # MTB[t][c][k][c3] = D1[9k+3t+3+c3]*D1[9k+3t+6+c3]
                # MT[t][c][c3][k] planar: dense k-run writes -> 2x adds
                PW = ucn * 3 * K
                for t in (0, 1):
                    nc.vector.tensor_tensor(
                        bass.AP(MTB.tensor, MTB.offset + t * PW,
                                [MTB.ap[0], [3 * K, ucn], [K, 3], [1, K]]),
                        d1v(1 + t, [[NW, 3], [3, K]]),
                        d1v(2 + t, [[NW, 3], [3, K]]),
                        alu.mult)

                # DT[t][c][k] = sum_c3 MTB  (dense step-1 fp16, 2x)
                def mtp(c3):
                    return bass.AP(MTB.tensor, MTB.offset + c3 * K,
                                   [MTB.ap[0], [PW, 2], [3 * K, ucn], [1, K]])

                eng_a = nc.gpsimd if GP_DADD else nc.vector
                eng_a.tensor_tensor(TMD[:], mtp(0), mtp(1), alu.add)
                eng_a.tensor_tensor(DT[:], TMD[:], mtp(2), alu.add)

                # PCP[t][c][k] = N[3k+1+t]*N[3k+2+t]
                eng_p = nc.gpsimd if GP_PC else nc.vector
                def npv(off):
                    return bass.AP(N.tensor, N.offset + off,
                                   [N.ap[0], [1, 2], [NW, ucn], [3, K]])

                eng_p.tensor_tensor(PCP[:], npv(1), npv(2), alu.mult)

                # q -> rq -> tc in two halves so each Act rsqrt
                # overlaps the other half's DVE work
                for hb in (0, UNB):
                    nc.vector._custom_dve(
                        OPS["QSUB"], out=QQ[:, hb:hb + UNB],
                        in0=PCP[:, hb:hb + UNB], in1=DT[:, hb:hb + UNB],
                        s0=1e-9, s1=1.0)
                    nc.scalar.activation(RQ[:, hb:hb + UNB],
                                         QQ[:, hb:hb + UNB],
                                         AF.Abs_reciprocal_sqrt)
                    nc.vector.tensor_tensor(TC[:, hb:hb + UNB],
                                            DT[:, hb:hb + UNB],
                                            RQ[:, hb:hb + UNB], alu.mult)
                nc.scalar.activation(
                    RN[:].rearrange("p (c k) -> p c k", c=ucn),
                    nv(2, [[3, K]]), AF.Abs_reciprocal_sqrt)
                # blen score while still on table 0's phase
                nc.vector._custom_dve(OPS["WMIN0B"], out=ACC[:],
                                      in0=nv(2, [[3, K]]), in1=RN[:],
                                      s0=a0s, s1=b0, imm2=C0)
                # gate chain: one Square(scale=0) per abs_rsqrt output; RQ
                # last so the final write keys off the tail-critical op
                nc.scalar.activation(gate[:], RN[:, 0:1], AF.Square, scale=0.0)
                nc.scalar.activation(gate[:], RQ[:, 0:1], AF.Square, scale=0.0)

            # phase B: arctans read the gate as bias -> forced after all
            # abs_rsqrt work -> exactly one switch to the trig table
            with tc.high_priority(offset=-(1 << 20)):
                for u, coff, ucn, S, chunks in units:
                    TC, ACC = tcs[u], accs[u]
                    UNB = ucn * K
                    AR = wk.tile([128, 2 * UNB], f16, tag=f"AR{u}")
                    def arv(t0):
                        return bass.AP(AR.tensor, AR.offset + t0 * UNB,
                                       [AR.ap[0], [1, UNB]])
                    # per-plane arctans so each WMIN overlaps the next
                    # arctan.  plane 1 = dot(v1,v2) -> ang_cnca (a1);
                    # plane 0 = dot(v3,v1) -> ang_cacn, sign in -a2
                    nc.scalar.activation(AR[:, UNB:], TC[:, UNB:],
                                         AF.Arctan, bias=gate[:, 0:1])
                    nc.vector._custom_dve(OPS["WMIN"], out=ACC[:],
                                          in0=arv(1), in1=ACC[:],
                                          s0=a1, s1=b1, imm2=C1)
                    nc.scalar.activation(AR[:, :UNB], TC[:, :UNB],
                                         AF.Arctan, bias=gate[:, 0:1])
                    nc.vector._custom_dve(OPS["WMIN"], out=ACC[:],
                                          in0=arv(0), in1=ACC[:],
                                          s0=-a2, s1=b2, imm2=C2)
                    nc.sync.dma_start(
                        bass.AP(out, coff * 128 * K, [[UNB, 128], [1, UNB]]),
                        ACC[:])
    nc.compile()
    return nc


# --------------------------------------------------------------------- run
def _in_maps(coords):
    """Repack coords into the per-core SBUF slab layout [g][p][c][j(585)],
    pre-scaled by PRE so fp16 intermediates cannot overflow."""
    from numpy.lib.stride_tricks import as_strided

    cf = np.ascontiguousarray(coords, dtype=np.float32).reshape(-1)
    cf = np.concatenate([cf, np.full(16, 1.0, dtype=np.float32)])
    s = cf.itemsize
    in_maps = []
    for i in range(NCORES):
        base = cf[i * CORE_F:]
        v = as_strided(base, shape=(BPC, 128, CH, CW),
                       strides=(GRP_F * s, 576 * s, CHAIN_F * s, s))
        w = (v * np.float32(PRE)).reshape(BPC, 128, CH, CW // 3, 3)
        in_maps.append({"sarr": np.ascontiguousarray(
            w.swapaxes(-1, -2)).reshape(-1)})
    return in_maps


def _unshard(outs):
    """outs: per-core fp16 unit-major arrays -> full [B, CH, R, NALT].
    Device layout: unit A (chains 0..11) then unit B (chains 12..15),
    each [128, ucn*K]; chain index = batch*CH + ch."""
    full = np.zeros((B, CH, R, NALT), dtype=np.float32)
    for i, o in enumerate(outs):
        o = np.asarray(o)
        a = o[: 128 * 8 * K].reshape(128, 8, K)
        b = o[128 * 8 * K:].reshape(128, 8, K)
        v = np.concatenate([a, b], axis=1)          # [p][c16][k]
        v = v.transpose(1, 0, 2).reshape(2 * CH, R)  # [c16][r=64p+k]
        for g in range(BPC):
            full[2 * i + g, :, : R - 1, 0] = (
                v[g * CH:(g + 1) * CH, : R - 1].astype(np.float32))
    return full


def _run_fast(coords, consts):
    from concourse.bass_utils import run_bass_kernel_spmd

    if consts not in _BUILT:
        _BUILT[consts] = _build(consts)
    nc = _BUILT[consts]
    res = run_bass_kernel_spmd(nc, _in_maps(coords), core_ids=list(range(NCORES)))
    return _unshard([r["out"] for r in res.results])


def _reference_numpy(atom_description, coords, alternatives, weight, mean, std):
    """Pure-numpy mirror of the jax reference (general-input fallback)."""
    ad = np.asarray(atom_description)
    coords = np.asarray(coords, dtype=np.float32)
    at, resnum, chain, batch, resname = (ad[:, i] for i in range(5))
    n = coords.shape[0]
    table = np.full((B, CH, R, 3), -1, dtype=np.int32)
    table[batch, chain, resnum, at] = np.arange(n, dtype=np.int32)

    c_idx = table[:, :, :-1, 2].reshape(-1)
    n_idx = table[:, :, 1:, 0].reshape(-1)
    cac_idx = table[:, :, :-1, 1].reshape(-1)
    can_idx = table[:, :, 1:, 1].reshape(-1)
    valid = (c_idx >= 0) & (n_idx >= 0) & (cac_idx >= 0) & (can_idx >= 0)

    safe = lambda i: np.where(i >= 0, i, 0)
    cc = coords[safe(c_idx)]
    ncrd = coords[safe(n_idx)]
    cacc = coords[safe(cac_idx)]
    canc = coords[safe(can_idx)]

    def angle_deg(a, b):
        na = np.linalg.norm(a, axis=-1).astype(np.float32)
        nb = np.linalg.norm(b, axis=-1).astype(np.float32)
        mask = (na > 0) & (nb > 0)
        cosang = np.sum(a * b, axis=-1) / np.maximum(na * nb, np.float32(1e-12))
        ang = np.degrees(np.arccos(np.clip(cosang, -1.0, 1.0))).astype(np.float32)
        return ang, mask

    blen = np.linalg.norm(cc - ncrd, axis=-1).astype(np.float32)
    v_cn = ncrd - cc
    ang1, m1 = angle_deg(v_cn, canc - ncrd)
    ang2, m2 = angle_deg(cc - cacc, -v_cn)
    valid = valid & m1 & m2

    x = np.stack([blen, ang1, ang2], axis=-1)
    seq = resname[safe(c_idx)]
    mu = np.asarray(mean, np.float32)[seq]
    var = np.asarray(std, np.float32)[seq] ** 2
    denom = np.sqrt(2.0 * np.pi * var).astype(np.float32)
    pdf = np.exp(-((x - mu) ** 2) / (2.0 * var)) / denom
    score = -(np.log(np.maximum(pdf, np.float32(EPS))) + np.log(denom))
    total = score.sum(-1) * (1.0 - np.tanh(-np.asarray(weight, np.float32)[0]))
    total = np.where(valid, total, np.float32(0.0)).astype(np.float32)

    resi = np.zeros((B, CH, R, NALT), dtype=np.float32)
    resi[:, :, : R - 1, 0] = total.reshape(B, CH, R - 1)
    return resi


def kernel(atom_description, coords, alternatives, weight, mean, std):
    if _check_structured(atom_description, coords, mean, std, weight):
        consts = _consts(mean, std, weight)
        if consts is not None:
            return _run_fast(coords, consts)
    return _reference_numpy(atom_description, coords, alternatives, weight, mean, std)
